# revision 2
# baseline (speedup 1.0000x reference)
"""Trainium2 Bass kernel for nn_Conv3x3 (3x3 conv, stride 3 == kernel, no overlap).

Math (reduced from the switched-capacitor reference):
    out[w, h] = -(1/0.924458) * sum_{i,j} x[3w+i, 3h+j] * weight[i, j]
    returned flattened to (2048*2048,), row-major over (w, h).

Strategy: data-parallel over x rows, 8 cores x 768 rows each. Per core,
7 row-blocks (6 x 126 rows + 1 x 12 rows). Each block is one contiguous
DMA (126, 6144). The whole reduction runs on the TensorEngine: for each
column-phase j in {0,1,2}, a band-structured weight matrix wband_j
(built on host from the 9 weight values, scale folded in) is the
stationary operand, and the x tile with a stride-3 column access pattern
is the moving operand; the 3 phases accumulate in PSUM. ScalarE copies
PSUM->SBUF, DMA stores to the output.

Self-contained: hardcodes shapes/sharding for x=(6144,6144) f32,
weight=(3,3) f32 on 8 NeuronCores.
"""

import numpy as np

INIT_C1_SCALED = 0.924458
SCALE = -1.0 / INIT_C1_SCALED

NCORES = 8
SIDE = 6144            # x is (SIDE, SIDE)
ROWS_PER_CORE = SIDE // NCORES      # 768
OUT_ROWS = SIDE // 3                # 2048
OUT_COLS = SIDE // 3                # 2048
OUT_ROWS_PER_CORE = ROWS_PER_CORE // 3   # 256

BLK_ROWS = 126         # x rows per block (multiple of 3, <= 128)
BLK_OUT = BLK_ROWS // 3             # 42 output rows per block
N_CHUNK = 512          # matmul moving-dim chunk (fp32 max)
N_CHUNKS = OUT_COLS // N_CHUNK      # 4

# per-core blocks: 6 full (126 rows) + 1 remainder (12 rows -> 4 out rows)
FULL_BLOCKS = ROWS_PER_CORE // BLK_ROWS          # 6
REM_ROWS = ROWS_PER_CORE - FULL_BLOCKS * BLK_ROWS    # 12
REM_OUT = REM_ROWS // 3                              # 4

_PREPARED = {}


def _split_excess_waits(nc, mybir, max_main=1):
    """walrus in this env rejects >1 sync wait per instruction; spill
    extras onto same-engine NoOps placed immediately before."""
    counter = 0
    for f in nc.m.functions:
        for bb in f.blocks:
            new = []
            changed = False
            for ins in bb.instructions:
                si = ins.sync_info
                waits = list(si.on_wait) if si and si.on_wait else []
                if len(waits) > max_main:
                    for w in waits[:-max_main]:
                        nop = mybir.InstNoOp(name=f"I-wsplit-{counter}")
                        counter += 1
                        nop.engine = ins.engine
                        nop.sync_info = mybir.SyncInfo(on_wait=[w], on_update=[])
                        new.append(nop)
                    ins.sync_info = mybir.SyncInfo(
                        on_wait=waits[-max_main:],
                        on_update=list(si.on_update) if si.on_update else [],
                    )
                    changed = True
                new.append(ins)
            if changed:
                bb.instructions = new


def build_program(reps=1):
    """Build the SPMD Bass program (one NeuronCore's slab). reps>1 repeats
    the whole body for marginal-timing runs."""
    import concourse.bass as bass
    import concourse.tile as tile
    from concourse import mybir

    f32 = mybir.dt.float32

    nc = bass.Bass("TRN2", target_bir_lowering=False, debug=False)
    xs = nc.dram_tensor("xs", [ROWS_PER_CORE, SIDE], f32, kind="ExternalInput").ap()
    wb = nc.dram_tensor("wb", [BLK_ROWS, 3 * BLK_OUT], f32, kind="ExternalInput").ap()
    out = nc.dram_tensor(
        "out", [OUT_ROWS_PER_CORE, OUT_COLS], f32, kind="ExternalOutput"
    ).ap()

    with tile.TileContext(nc) as tc:
        with (
            tc.tile_pool(name="wpool", bufs=1) as wpool,
            tc.tile_pool(name="xpool", bufs=4) as xpool,
            tc.tile_pool(name="opool", bufs=3) as opool,
            tc.tile_pool(name="pspool", bufs=2, space="PSUM") as pspool,
        ):
            wt = wpool.tile([BLK_ROWS, 3 * BLK_OUT], f32)
            nc.sync.dma_start(wt[:], wb)

            for _ in range(reps):
                for b in range(FULL_BLOCKS + 1):
                    nr = BLK_ROWS if b < FULL_BLOCKS else REM_ROWS
                    m = BLK_OUT if b < FULL_BLOCKS else REM_OUT
                    if nr == 0:
                        continue
                    xt = xpool.tile([BLK_ROWS, SIDE], f32, tag="xt")
                    nc.sync.dma_start(
                        xt[0:nr, :], xs[b * BLK_ROWS : b * BLK_ROWS + nr, :]
                    )
                    pt = pspool.tile([BLK_OUT, OUT_COLS], f32, tag="pt")
                    for c in range(N_CHUNKS):
                        base = 3 * N_CHUNK * c
                        for j in range(3):
                            nc.tensor.matmul(
                                pt[0:m, c * N_CHUNK : (c + 1) * N_CHUNK],
                                wt[0:nr, j * BLK_OUT : j * BLK_OUT + m],
                                xt[0:nr, base + j : base + j + 3 * (N_CHUNK - 1) + 1 : 3],
                                start=(j == 0),
                                stop=(j == 2),
                            )
                    ot = opool.tile([BLK_OUT, OUT_COLS], f32, tag="ot")
                    nc.scalar.copy(ot[0:m, :], pt[0:m, :])
                    nc.sync.dma_start(out[b * BLK_OUT : b * BLK_OUT + m, :], ot[0:m, :])

    _split_excess_waits(nc, mybir)
    return nc


def build_wband(weight):
    """wband[p, 42*j + w'] = SCALE * weight[p%3, j] if p//3 == w' else 0."""
    wband = np.zeros((BLK_ROWS, 3 * BLK_OUT), np.float32)
    w = np.asarray(weight, dtype=np.float32)
    for p in range(BLK_ROWS):
        i, wp = p % 3, p // 3
        for j in range(3):
            wband[p, BLK_OUT * j + wp] = SCALE * w[i, j]
    return wband


def kernel(x, weight):
    from concourse.bass_utils import run_bass_kernel_spmd

    x = np.ascontiguousarray(np.asarray(x, dtype=np.float32))
    assert x.shape == (SIDE, SIDE)
    wband = build_wband(weight)

    if "nc" not in _PREPARED:
        _PREPARED["nc"] = build_program()
    nc = _PREPARED["nc"]

    in_maps = [
        {"xs": x[c * ROWS_PER_CORE : (c + 1) * ROWS_PER_CORE], "wb": wband}
        for c in range(NCORES)
    ]
    res = run_bass_kernel_spmd(nc, in_maps, list(range(NCORES)))
    out = np.concatenate(
        [res.results[c]["out"].reshape(-1) for c in range(NCORES)]
    )
    return out
